# revision 1
# baseline (speedup 1.0000x reference)
"""Trainium2 Bass kernel for nn_MemoryBlock (batched LSTM scan with reset gating).

Problem (hardcoded shapes):
  bs=512, na=64, seq_len=16, nt=32, H=512, N_ATTN=256.
  x = concat(h_self[:,:,256:], h_inter, -1)            -> [512, 64, 512]
  time-major X: [16, 2048, 512]; LSTM cell per step with
  h,c reset-masked by (1-reset) before the cell. Outputs all
  intermediate h,c states, remapped back to [512, 64, 512].

Strategy: data-parallel over the 2048-row step-batch, 256 rows/core on 8
cores; small LSTM weights replicated. All layout transforms (time-major
reorder, feature-major transposes, weight pre-transposition, reset-mask
replication) are done host-side in numpy, so the device kernel is a pure
fused-matmul recurrence:

  per step t: gates.T [2048, 256] = W_comb.T.T @ [x_t; h_{t-1}].T
  accumulated in PSUM over K=1024 (8 chunks of 128: 4 x-chunks then 4
  h-chunks), one PSUM bank per 2 gate-feature chunks (8 banks/step).
  The x-part of step t+1 sits between h-parts of t and t+1 in the PE
  stream, hiding the ACT/DVE cell latency so PE never stalls.

Matmul operands are bf16 (fp32 matmuls are self-loading single-wait
instructions this walrus rejects with Tile's multi-waits, and bf16 enables
fast weight load); PSUM accumulation and all cell math stay fp32
(measured rel err ~2.3e-3 end to end).

Layouts (per core), feature-major "T" = [feature-on-partition, batch]:
  w   [128, 16384] bf16: w[p, (k*16+m)*128+q] = W_comb[128m+q, 128k+p],
                         W_comb = [W_ih | W_hh] (2048 x 1024)
  x   [16, 128, 4, 256] bf16: x[t, p, kc, b] = X[t, row b, 128*kc+p]
  m   [16, 128, 256] bf16: (1-reset) replicated over partitions
  h0,c0 [128, 4, 256] f32: initial states, feature-major
  hys,cys [16, 128, 4, 256] f32 outputs, feature-major (host transposes back)
"""

import sys

import numpy as np

sys.path.insert(0, "/opt/pypackages")
sys.path.insert(0, "/opt/trn_rl_repo")

import concourse.bass as bass  # noqa: E402
import concourse.bacc as bacc  # noqa: E402
import concourse.mybir as mybir  # noqa: E402
import concourse.tile as tile  # noqa: E402

SEQ = 16
NT = 32
NA = 64
H = 512
N_ATTN = 256
BS = NT * SEQ  # 512
BATCH = NT * NA  # 2048
N_CORES = 8
RPC = BATCH // N_CORES  # 256 rows per core
F32 = mybir.dt.float32
BF16 = mybir.dt.bfloat16

_CACHE = {}


def _build_bass():
    """Build the single-core Bass program (same NEFF runs SPMD on 8 cores)."""
    nc = bacc.Bacc(None, target_bir_lowering=False)

    w_d = nc.dram_tensor("w", [128, 8 * 16 * 128], BF16, kind="ExternalInput")
    x_d = nc.dram_tensor("x", [SEQ, 128, 4, 256], BF16, kind="ExternalInput")
    m_d = nc.dram_tensor("m", [SEQ, 128, 256], BF16, kind="ExternalInput")
    h0_d = nc.dram_tensor("h0", [128, 4, 256], F32, kind="ExternalInput")
    c0_d = nc.dram_tensor("c0", [128, 4, 256], F32, kind="ExternalInput")
    hys_d = nc.dram_tensor("hys", [SEQ, 128, 4, 256], F32, kind="ExternalOutput")
    cys_d = nc.dram_tensor("cys", [SEQ, 128, 4, 256], F32, kind="ExternalOutput")

    SIG = mybir.ActivationFunctionType.Sigmoid
    TANH = mybir.ActivationFunctionType.Tanh

    with tile.TileContext(nc) as tc:
        with (
            tc.tile_pool(name="const", bufs=1) as const,
            tc.tile_pool(name="xin", bufs=4) as xin,
            tc.tile_pool(name="min", bufs=4) as min_,
            tc.tile_pool(name="state", bufs=2) as state,
            tc.tile_pool(name="gates", bufs=2) as gpool,
            tc.tile_pool(name="psum", bufs=8, space="PSUM") as psum,
        ):
            # DMA bandwidth is a shared serial resource; emit transfers in
            # consumption order so the first matmuls start after ~1MB, not
            # after the full 4.2MB of weights: x0, W0-3 (x-part of step 0),
            # x1, W4-7 (h-part), then state/masks.
            def load_x(t):
                x4 = xin.tile([128, 4, 256], BF16, tag="x", name=f"x{t}")
                nc.sync.dma_start(x4[:], x_d[t])
                return x4

            w0a = const.tile([128, 4 * 128], BF16, tag="W0a", name="W0a")
            nc.sync.dma_start(w0a[:], w_d[:, 0 : 4 * 128])
            x0p = []
            for q in range(4):
                xp = xin.tile([128, 256], BF16, tag=f"x0p{q}", name=f"x0p{q}",
                              bufs=1)
                nc.sync.dma_start(xp[:], x_d[0, :, q])
                x0p.append(xp)
            x_tiles = {}
            w0b = const.tile([128, 12 * 128], BF16, tag="W0b", name="W0b")
            nc.sync.dma_start(w0b[:], w_d[:, 4 * 128 : 2048])
            Wk = [None] * 8
            for k in range(1, 4):
                wt = const.tile([128, 16 * 128], BF16, tag=f"W{k}", name=f"W{k}")
                nc.sync.dma_start(wt[:], w_d[:, k * 2048 : (k + 1) * 2048])
                Wk[k] = wt
            def load_m(t):
                m = min_.tile([128, 256], BF16, tag="m", name=f"m{t}")
                nc.gpsimd.dma_start(m[:], m_d[t])
                return m

            def load_w(k):
                wt = const.tile([128, 16 * 128], BF16, tag=f"W{k}", name=f"W{k}")
                nc.sync.dma_start(wt[:], w_d[:, k * 2048 : (k + 1) * 2048])
                Wk[k] = wt

            # Initial state, one tile per feature-pair half so every
            # downstream dependency is at half granularity. Interleaved with
            # W4-7 so the h-part weights don't queue behind all of the
            # state/mask bytes in the DMA pipe.
            # SWDGE lanes: any DVE op depending on several of these waits
            # on few sems, staying under walrus's one-sync-wait-per-
            # instruction limit (a DVE "touch" below funnels the mask sem).
            h_prev, c_prev = [], []
            load_w(4)
            for v in range(2):
                hp = state.tile([128, 2, 256], F32, tag=f"h{v}", name=f"h_init{v}")
                cp = state.tile([128, 2, 256], F32, tag=f"c{v}", name=f"c_init{v}")
                nc.gpsimd.dma_start(hp[:], h0_d[:, 2 * v : 2 * v + 2])
                nc.gpsimd.dma_start(cp[:], c0_d[:, 2 * v : 2 * v + 2])
                h_prev.append(hp)
                c_prev.append(cp)
                load_w(5 + v)
            m_tiles = {0: load_m(0)}
            load_w(7)
            m_tiles[1] = load_m(1)
            x_tiles[1] = load_x(1)
            x_tiles[2] = load_x(2)

            def lhsT(k, mi):
                if k == 0:
                    if mi < 4:
                        return w0a[:, mi * 128 : (mi + 1) * 128]
                    return w0b[:, (mi - 4) * 128 : (mi - 3) * 128]
                return Wk[k][:, mi * 128 : (mi + 1) * 128]

            for t in range(SEQ):
                if 3 <= t + 3 < SEQ:
                    x_tiles[t + 3] = load_x(t + 3)
                if 2 <= t + 2 < SEQ:
                    m_tiles[t + 2] = load_m(t + 2)
                if t == 0:
                    xt = [x0p[kc][:] for kc in range(4)]
                else:
                    xt4 = x_tiles.pop(t)
                    xt = [xt4[:, kc, :] for kc in range(4)]
                mt = m_tiles.pop(t)
                m_b = mt[:].unsqueeze(1).broadcast_to([128, 2, 256])

                # Touch mt with a 1-element DVE copy so the mask-muls below
                # never carry two DMA sem waits (walrus allows one sync wait
                # per compute instruction).
                sc = state.tile([128, 1], F32, tag="sc")
                nc.vector.tensor_copy(sc[:], mt[:, :1])

                # Reset-mask previous state (DVE). hm feeds the matmul rhs.
                hm, cm = [], []
                for v in range(2):
                    hmv = state.tile([128, 2, 256], BF16, tag=f"hm{v}",
                                     name=f"hm{t}_{v}")
                    cmv = state.tile([128, 2, 256], F32, tag=f"cm{v}",
                                     name=f"cm{t}_{v}")
                    nc.vector.tensor_mul(hmv[:], h_prev[v][:], m_b)
                    nc.vector.tensor_mul(cmv[:], c_prev[v][:], m_b)
                    hm.append(hmv)
                    cm.append(cmv)

                # 8 PSUM banks: bank j holds gate-feature chunks (2j, 2j+1)
                # for the full 256-row batch -> [128, 2, 256].
                banks = [
                    psum.tile([128, 2, 256], F32, tag="bank", name=f"bank{t}_{j}")
                    for j in range(8)
                ]

                # x-part: K-chunks 0..3 (only needs xt) - PE does this while
                # the previous step's cell math is still in flight.
                for k in range(4):
                    rhs = xt[k][:]
                    for j in range(8):
                        for u in range(2):
                            mi = 2 * j + u
                            # One accumulation group per bank (zero region =
                            # full bank): start only on the bank's first MM.
                            nc.tensor.matmul(
                                banks[j][:, u, :],
                                lhsT(k, mi),
                                rhs,
                                start=(k == 0 and u == 0),
                                stop=False,
                            )

                # h-part: K-chunks 4..7, bank-major (g first, then i, f, o so
                # the cell's critical operands are ready earliest). ACT
                # evacuates each bank into its own per-(gate, half) tile.
                gsb = {}
                for j in (4, 5, 0, 1, 2, 3, 6, 7):
                    for u in range(2):
                        mi = 2 * j + u
                        for k in range(4, 8):
                            kc = k - 4
                            nc.tensor.matmul(
                                banks[j][:, u, :],
                                lhsT(k, mi),
                                hm[kc // 2][:, kc % 2, :],
                                start=False,
                                stop=(k == 7 and u == 1),
                            )
                    # banks 0,1 -> i (sigmoid); 2,3 -> f; 4,5 -> g (tanh);
                    # 6,7 -> o.
                    g_, half = j // 2, j % 2
                    func = TANH if g_ == 2 else SIG
                    gt = gpool.tile([128, 2, 256], F32, tag=f"g{g_}_{half}",
                                    name=f"g{t}_{g_}_{half}")
                    nc.scalar.activation(gt[:], banks[j][:], func)
                    gsb[(g_, half)] = gt

                # Cell math (DVE) + tanh (ACT), independent per half.
                h_new, c_new = [], []
                for v in range(2):
                    ig = state.tile([128, 2, 256], F32, tag=f"ig{v}",
                                    name=f"ig{t}_{v}")
                    nc.vector.tensor_mul(ig[:], gsb[(0, v)][:], gsb[(2, v)][:])
                    cn = state.tile([128, 2, 256], F32, tag=f"c{v}",
                                    name=f"c{t}_{v}")
                    nc.vector.tensor_mul(cn[:], gsb[(1, v)][:], cm[v][:])
                    nc.vector.tensor_add(cn[:], cn[:], ig[:])
                    th = state.tile([128, 2, 256], F32, tag=f"th{v}",
                                    name=f"th{t}_{v}")
                    nc.scalar.activation(th[:], cn[:], TANH)
                    hn = state.tile([128, 2, 256], F32, tag=f"h{v}",
                                    name=f"h{t}_{v}")
                    nc.vector.tensor_mul(hn[:], gsb[(3, v)][:], th[:])
                    nc.sync.dma_start(cys_d[t, :, 2 * v : 2 * v + 2], cn[:])
                    nc.sync.dma_start(hys_d[t, :, 2 * v : 2 * v + 2], hn[:])
                    h_new.append(hn)
                    c_new.append(cn)
                h_prev, c_prev = h_new, c_new

    nc.compile()
    return nc


def _get_nc():
    if "nc" not in _CACHE:
        _CACHE["nc"] = _build_bass()
    return _CACHE["nc"]


def _prep_inputs(h_self, h_inter, hxs, cxs, reset, W_ih, W_hh, b_ih, b_hh):
    """Host-side layout transforms -> list of per-core input dicts."""
    f = np.float32
    x = np.concatenate([h_self[:, :, N_ATTN:], h_inter], axis=-1).astype(f)  # [512,64,512]
    # time-major [16, 2048, 512]
    x_tm = np.ascontiguousarray(
        x.reshape(NT, SEQ, NA, H).transpose(1, 0, 2, 3).reshape(SEQ, BATCH, H)
    )
    resets = np.broadcast_to(reset.astype(f), (BS, NA))
    resets_tm = resets.reshape(NT, SEQ, NA).transpose(1, 0, 2).reshape(SEQ, BATCH)
    mask_tm = (1.0 - resets_tm).astype(f)
    h0 = hxs[::SEQ].reshape(BATCH, H).astype(f)
    c0 = cxs[::SEQ].reshape(BATCH, H).astype(f)

    assert not np.any(b_ih) and not np.any(b_hh), "nonzero LSTM bias unsupported"

    # Weights: W_comb = [W_ih | W_hh] [2048, 1024]; A = W_comb.T [1024, 2048]
    # w[p, (k*16+m)*128+q] = A[128k+p, 128m+q]
    import ml_dtypes
    bf16 = ml_dtypes.bfloat16
    A = np.concatenate([W_ih, W_hh], axis=1).T.astype(f)  # [1024, 2048]
    w = np.ascontiguousarray(
        A.reshape(8, 128, 16, 128).transpose(1, 0, 2, 3).reshape(128, 8 * 16 * 128)
    ).astype(bf16)

    in_maps = []
    for c in range(N_CORES):
        rows = slice(c * RPC, (c + 1) * RPC)
        xc = x_tm[:, rows, :]  # [16, 256, 512]
        # x[t, p, kc, b] = xc[t, b, 128*kc+p]
        xd = np.ascontiguousarray(
            xc.reshape(SEQ, RPC, 4, 128).transpose(0, 3, 2, 1)
        ).astype(bf16)  # [16, 128, 4, 256]
        md = np.ascontiguousarray(
            np.broadcast_to(mask_tm[:, rows][:, None, :], (SEQ, 128, RPC))
        ).astype(bf16)
        h0d = np.ascontiguousarray(h0[rows].reshape(RPC, 4, 128).transpose(2, 1, 0))
        c0d = np.ascontiguousarray(c0[rows].reshape(RPC, 4, 128).transpose(2, 1, 0))
        in_maps.append({"w": w, "x": xd, "m": md, "h0": h0d, "c0": c0d})
    return in_maps


def _postprocess(results):
    """Per-core feature-major outputs -> full [512, 64, 512] hys, cys."""
    outs = []
    for key in ("hys", "cys"):
        tm = np.empty((SEQ, BATCH, H), dtype=np.float32)
        for c, res in enumerate(results):
            rows = slice(c * RPC, (c + 1) * RPC)
            # res[key][t, p, kc, b] -> tm[t, row b, 128*kc+p]
            tm[:, rows, :] = (
                res[key].transpose(0, 3, 2, 1).reshape(SEQ, RPC, H)
            )
        out = tm.reshape(SEQ, NT, NA, H).transpose(1, 0, 2, 3).reshape(BS, NA, H)
        outs.append(np.ascontiguousarray(out))
    return outs[0], outs[1]


def kernel(h_self, h_inter, hxs, cxs, reset, W_ih, W_hh, b_ih, b_hh, seq_len,
           trace=False, tmpdir=None):
    assert int(seq_len) == SEQ
    from concourse.bass_utils import run_bass_kernel_spmd

    nc = _get_nc()
    in_maps = _prep_inputs(
        np.asarray(h_self), np.asarray(h_inter), np.asarray(hxs), np.asarray(cxs),
        np.asarray(reset), np.asarray(W_ih), np.asarray(W_hh),
        np.asarray(b_ih), np.asarray(b_hh),
    )
    res = run_bass_kernel_spmd(
        nc, in_maps, core_ids=list(range(N_CORES)), trace=trace, tmpdir=tmpdir
    )
    _CACHE["last_results"] = res
    return _postprocess(res.results)



# revision 7
# speedup vs baseline: 1.8077x; 1.8077x over previous
"""Trainium2 Bass kernel for nn_MemoryBlock (batched LSTM scan with reset gating).

Problem (hardcoded shapes):
  bs=512, na=64, seq_len=16, nt=32, H=512, N_ATTN=256.
  x = concat(h_self[:,:,256:], h_inter, -1)            -> [512, 64, 512]
  time-major X: [16, 2048, 512]; LSTM cell per step with
  h,c reset-masked by (1-reset) before the cell. Outputs all
  intermediate h,c states, remapped back to [512, 64, 512].

Strategy: data-parallel over the 2048-row step-batch, 256 rows/core on 8
cores; weights replicated. Per core the batch splits into 2 independent
128-column streams so the recurrence latency of one stream hides behind the
other's engine work.

Matmuls run as fp8e4 DoubleRow (2 K-tiles of 128 per instruction, 0.5
cycles/row): weights are scaled x128 and split hi+residual; per-gate product
counts are chosen by error sensitivity (validated vs the fp32 reference,
rel err ~1.4e-2 < 2e-2):
  g (tanh, slope 1):   x_hi@W_hi + x_lo@W_hi + x_hi@W_lo   (3 products)
  i, f (sigmoid):      x_hi@W_hi + x_hi@W_lo                (2 products)
  o (sigmoid):         x_hi@W_hi                            (1 product)
  h-part (all gates):  h_fp8@W_hh_hi                        (1 product)
Residuals are stored unscaled (subnormal-heavy but only on small elements,
whose products are negligible). PSUM accumulates fp32; the x128 weight scale
is divided out by the activation's scale=1/128.

Gate order in PSUM is permuted to [g, i, f, o] so each stream evacuates with
two ACT instructions: tanh over the g bank, sigmoid over the i/f/o banks
(3 contiguous banks, one instruction). Cell math and reset-mask muls are
bf16 on DVE (2x_1p); h re-quantizes to fp8 for the next step's matmul.
Outputs stream out as bf16 over the SWDGE path (gpsimd dma_start) so store
descriptor generation stays off the HWDGE ring that feeds the loads; the
host converts back to f32.

Layouts (per core), feature-major "T" = [feature-on-partition, batch]:
  wh   [128, 64, 2, 128] fp8: wh[p, 16*pair+mi, j, q] = A[128*(2pair+j)+p, 128*mi+q]
       A = 128 * [W_ih | W_hh].T with gate columns permuted to (g,i,f,o)
  wxlo [128, 24, 2, 128] fp8: same for the x-row residual of A, gates g,i,f only
  x8   [16, 128, 2, 4, 256] fp8: x hi/lo terms, x8[t, p, e, kc, b] = X_e[t, row b, 128kc+p]
  m2   [8, 128, 2, 256] bf16: (1-reset) replicated over partitions, step pairs
  h0, c0 [128, 4, 256] bf16: initial states, feature-major
  hys, cys [16, 2, 128, 4, 128] bf16: outputs, stream-major (host transposes back)
"""

import sys

import numpy as np

sys.path.insert(0, "/opt/pypackages")
sys.path.insert(0, "/opt/trn_rl_repo")

import concourse.bass as bass  # noqa: E402,F401
import concourse.bacc as bacc  # noqa: E402
import concourse.mybir as mybir  # noqa: E402
import concourse.tile as tile  # noqa: E402

SEQ = 16
NT = 32
NA = 64
H = 512
N_ATTN = 256
BS = NT * SEQ  # 512
BATCH = NT * NA  # 2048
N_CORES = 8
RPC = BATCH // N_CORES  # 256 rows per core
WS = 128.0  # weight pre-scale, divided out in the activation
F32 = mybir.dt.float32
BF16 = mybir.dt.bfloat16
FP8 = mybir.dt.float8e4
DR = mybir.MatmulPerfMode.DoubleRow

_CACHE = {}


def _build_bass():
    """Build the single-core Bass program (same NEFF runs SPMD on 8 cores)."""
    nc = bacc.Bacc(None, target_bir_lowering=False)

    wh_d = nc.dram_tensor("wh", [128, 64, 2, 128], FP8, kind="ExternalInput")
    wxlo_d = nc.dram_tensor("wxlo", [128, 24, 2, 128], FP8, kind="ExternalInput")
    x8_d = nc.dram_tensor("x8", [SEQ, 128, 2, 4, 256], FP8, kind="ExternalInput")
    m2_d = nc.dram_tensor("m2", [SEQ // 2, 128, 2, 256], BF16, kind="ExternalInput")
    h0_d = nc.dram_tensor("h0", [128, 4, 256], BF16, kind="ExternalInput")
    c0_d = nc.dram_tensor("c0", [128, 4, 256], BF16, kind="ExternalInput")
    hys_d = nc.dram_tensor("hys", [SEQ, 2, 128, 4, 128], BF16, kind="ExternalOutput")
    cys_d = nc.dram_tensor("cys", [SEQ, 2, 128, 4, 128], BF16, kind="ExternalOutput")

    SIG = mybir.ActivationFunctionType.Sigmoid
    TANH = mybir.ActivationFunctionType.Tanh

    with tile.TileContext(nc) as tc:
        with (
            tc.tile_pool(name="const", bufs=1) as const,
            tc.tile_pool(name="xin", bufs=3) as xin,
            tc.tile_pool(name="min", bufs=4) as min_,
            tc.tile_pool(name="state", bufs=2) as state,
            tc.tile_pool(name="gates", bufs=2) as gpool,
            tc.tile_pool(name="psum", bufs=1, space="PSUM") as psum,
        ):
            # --- preamble DMAs, in first-consumption order -----------------
            x_tiles, m_pairs = {}, {}

            def load_x(t):
                a = xin.tile([128, 2, 4, 256], FP8, tag="x8", name=f"x8_{t}")
                nc.sync.dma_start(a[:], x8_d[t])
                x_tiles[t] = a

            def load_m(pair):
                m = min_.tile([128, 2, 256], BF16, tag="m", name=f"m{pair}")
                nc.sync.dma_start(m[:], m2_d[pair])
                m_pairs[pair] = m

            def m_ap(t):
                return m_pairs[t // 2][:, t % 2]

            load_x(0)
            # weights: x-row pairs (0,1) first so step-0 x-matmuls start early
            wh = const.tile([128, 64, 2, 128], FP8, tag="wh", name="wh")
            wxlo = const.tile([128, 24, 2, 128], FP8, tag="wxlo", name="wxlo")
            nc.sync.dma_start(wh[:, 0:16], wh_d[:, 0:16])
            nc.sync.dma_start(wh[:, 16:32], wh_d[:, 16:32])
            nc.sync.dma_start(wxlo[:, 0:12], wxlo_d[:, 0:12])
            nc.sync.dma_start(wxlo[:, 12:24], wxlo_d[:, 12:24])
            load_m(0)
            h0 = state.tile([128, 4, 256], BF16, tag="h_init", name="h0", bufs=1)
            c0 = state.tile([128, 4, 256], BF16, tag="c_init", name="c0", bufs=1)
            nc.sync.dma_start(h0[:], h0_d[:])
            nc.sync.dma_start(c0[:], c0_d[:])
            nc.sync.dma_start(wh[:, 32:48], wh_d[:, 32:48])
            nc.sync.dma_start(wh[:, 48:64], wh_d[:, 48:64])
            load_x(1)
            load_m(1)
            load_x(2)

            def lw(pair, mi):
                return wh[:, 16 * pair + mi]  # [128, 2, 128] fp8

            def lwx(pair, mi):
                return wxlo[:, 12 * pair + mi]

            # --- per-step PSUM tiles & bank-group bookkeeping --------------
            # per stream: g tile = 1 bank (chunks g0-3), sio = 3 banks
            # (i0-3, f0-3, o0-3). bank key: (s, 0) for g, (s, 1+b) for sio.
            def new_psum(t):
                tiles = []
                for s in range(2):
                    g = psum.tile([128, 4, 128], F32, tag=f"gps{s}",
                                  name=f"gps{t}_{s}")
                    sio = psum.tile([128, 12, 128], F32, tag=f"sio{s}",
                                    name=f"sio{t}_{s}")
                    tiles.append((g, sio))
                return tiles

            def out_slot(tiles, s, mi):
                g, sio = tiles[s]
                if mi < 4:
                    return g[:, mi, :], (s, 0)
                return sio[:, mi - 4, :], (s, 1 + (mi - 4) // 4)

            # x-part matmuls for step t (both streams). Marks bank starts.
            def x_mms(t, tiles, started):
                xt = x_tiles.pop(t)
                xh_t, xl_t = xt[:, 0], xt[:, 1]
                for s in range(2):
                    cols = slice(128 * s, 128 * (s + 1))
                    for mi in range(16):
                        out, bank = out_slot(tiles, s, mi)
                        prods = [(lw, xh_t)]
                        if mi < 12:
                            prods.append((lwx, xh_t))
                        if mi < 4:
                            prods.append((lw, xl_t))
                        for wfn, rhs_t in prods:
                            for pair in range(2):
                                st = bank not in started
                                started.add(bank)
                                nc.tensor.matmul(
                                    out,
                                    wfn(pair, mi),
                                    rhs_t[:, 2 * pair : 2 * pair + 2, cols],
                                    start=st, stop=False, perf_mode=DR,
                                )

            # h-part matmuls for step t; g chunks first. Sets bank stops.
            def h_mms(t, tiles, hm):
                for s in range(2):
                    for mi in range(16):
                        out, bank = out_slot(tiles, s, mi)
                        last_in_bank = mi % 4 == 3
                        for pair in (2, 3):
                            nc.tensor.matmul(
                                out,
                                lw(pair, mi),
                                hm[s][:, 2 * (pair - 2) : 2 * (pair - 2) + 2, :],
                                start=False,
                                stop=(last_in_bank and pair == 3),
                                perf_mode=DR,
                            )

            # reset-mask h/c for step t (DVE), producing this step's hm/cm
            def mask_ops(t, h_prev, c_prev):
                hm, cm = [], []
                for s in range(2):
                    m_b = (m_ap(t)[:, 128 * s : 128 * (s + 1)]
                           .unsqueeze(1).broadcast_to([128, 4, 128]))
                    hp = h_prev[s] if isinstance(h_prev, list) else \
                        h_prev[:, :, 128 * s : 128 * (s + 1)]
                    cp = c_prev[s] if isinstance(c_prev, list) else \
                        c_prev[:, :, 128 * s : 128 * (s + 1)]
                    hmv = state.tile([128, 4, 128], FP8, tag=f"hm{s}",
                                     name=f"hm{t}_{s}")
                    cmv = state.tile([128, 4, 128], BF16, tag=f"cm{s}",
                                     name=f"cm{t}_{s}")
                    nc.vector.tensor_mul(hmv[:], hp, m_b)
                    nc.vector.tensor_mul(cmv[:], cp, m_b)
                    hm.append(hmv)
                    cm.append(cmv)
                return hm, cm

            # --- t=0: masks on initial state -------------------------------
            # touch m0 with a 1-elem DVE copy so the first mask-mul carries
            # a single DMA sem wait (walrus allows one sync wait per instr).
            tch = state.tile([128, 1], BF16, tag="tch", bufs=1)
            nc.vector.tensor_copy(tch[:], m_pairs[0][:, 0, :1])
            cur_psum = new_psum(0)
            started = set()
            x_mms(0, cur_psum, started)
            hm, cm = mask_ops(0, h0[:], c0[:])

            for t in range(SEQ):
                h_mms(t, cur_psum, hm)
                h_new, c_new = [], []
                for s in range(2):
                    g_ps, sio_ps = cur_psum[s]
                    gt = gpool.tile([128, 4, 128], BF16, tag=f"g{s}",
                                    name=f"g{t}_{s}")
                    nc.scalar.activation(gt[:], g_ps[:], TANH, scale=1.0 / WS)
                    sio = gpool.tile([128, 12, 128], BF16, tag=f"sio{s}",
                                     name=f"sio{t}_{s}")
                    nc.scalar.activation(sio[:], sio_ps[:], SIG, scale=1.0 / WS)
                    ig = state.tile([128, 4, 128], BF16, tag=f"ig{s}",
                                    name=f"ig{t}_{s}")
                    nc.vector.tensor_mul(ig[:], sio[:, 0:4], gt[:])
                    fcm = state.tile([128, 4, 128], BF16, tag=f"fcm{s}",
                                     name=f"fcm{t}_{s}")
                    nc.vector.tensor_mul(fcm[:], sio[:, 4:8], cm[s][:])
                    cn = state.tile([128, 4, 128], BF16, tag=f"cn{s}",
                                    name=f"c{t}_{s}")
                    nc.vector.tensor_add(cn[:], ig[:], fcm[:])
                    th = state.tile([128, 4, 128], BF16, tag=f"th{s}",
                                    name=f"th{t}_{s}")
                    nc.scalar.activation(th[:], cn[:], TANH)
                    hn = state.tile([128, 4, 128], BF16, tag=f"hn{s}",
                                    name=f"h{t}_{s}")
                    nc.vector.tensor_mul(hn[:], sio[:, 8:12], th[:])
                    nc.gpsimd.dma_start(cys_d[t, s], cn[:])
                    nc.gpsimd.dma_start(hys_d[t, s], hn[:])
                    h_new.append(hn[:])
                    c_new.append(cn[:])
                if t + 1 < SEQ:
                    hm, cm = mask_ops(t + 1, h_new, c_new)
                    if t + 3 < SEQ:
                        load_x(t + 3)
                    if t % 2 == 0 and (t + 4) // 2 < SEQ // 2:
                        load_m((t + 4) // 2)
                    nxt = new_psum(t + 1)
                    started = set()
                    x_mms(t + 1, nxt, started)
                    cur_psum = nxt

    nc.compile()
    return nc


def _get_nc():
    if "nc" not in _CACHE:
        _CACHE["nc"] = _build_bass()
    return _CACHE["nc"]


def _prep_inputs(h_self, h_inter, hxs, cxs, reset, W_ih, W_hh, b_ih, b_hh):
    """Host-side layout transforms -> list of per-core input dicts."""
    import ml_dtypes

    f = np.float32
    F8 = ml_dtypes.float8_e4m3
    bf16 = ml_dtypes.bfloat16

    x = np.concatenate([h_self[:, :, N_ATTN:], h_inter], axis=-1).astype(f)
    x_tm = np.ascontiguousarray(
        x.reshape(NT, SEQ, NA, H).transpose(1, 0, 2, 3).reshape(SEQ, BATCH, H)
    )
    resets = np.broadcast_to(reset.astype(f), (BS, NA))
    resets_tm = resets.reshape(NT, SEQ, NA).transpose(1, 0, 2).reshape(SEQ, BATCH)
    mask_tm = (1.0 - resets_tm).astype(f)
    h0 = hxs[::SEQ].reshape(BATCH, H).astype(f)
    c0 = cxs[::SEQ].reshape(BATCH, H).astype(f)

    assert not np.any(b_ih) and not np.any(b_hh), "nonzero LSTM bias unsupported"

    # A = 128 * [W_ih | W_hh].T [1024, 2048], gate columns permuted to g,i,f,o
    A = (np.concatenate([W_ih, W_hh], axis=1).T.astype(f) * WS)  # [1024, 2048]
    perm = np.concatenate([np.arange(2 * H, 3 * H),      # g
                           np.arange(0, H),              # i
                           np.arange(H, 2 * H),          # f
                           np.arange(3 * H, 4 * H)])     # o
    A = A[:, perm]
    A_hi8 = A.astype(F8)
    A_hi = A_hi8.astype(f)
    A_xlo8 = (A[:H] - A_hi[:H]).astype(F8)  # x rows residual
    # wh[p, 16*pair+mi, j, q] = A_hi[128*(2pair+j)+p, 128mi+q]
    wh = np.ascontiguousarray(
        A_hi8.reshape(4, 2, 128, 16, 128).transpose(2, 0, 3, 1, 4)
        .reshape(128, 64, 2, 128)
    )
    # wxlo: x pairs (0,1) x gates g,i,f (mi 0..11)
    wxlo = np.ascontiguousarray(
        A_xlo8[:, : 12 * 128].reshape(2, 2, 128, 12, 128).transpose(2, 0, 3, 1, 4)
        .reshape(128, 24, 2, 128)
    )

    x_hi8 = x_tm.astype(F8)
    x_lo8 = (x_tm - x_hi8.astype(f)).astype(F8)

    def xlayout(a):  # [16, RPC rows, 512] -> [16, 128, 4, 256]
        return a.reshape(SEQ, RPC, 4, 128).transpose(0, 3, 2, 1)

    in_maps = []
    for cix in range(N_CORES):
        rows = slice(cix * RPC, (cix + 1) * RPC)
        x8 = np.ascontiguousarray(np.stack(
            [xlayout(x_hi8[:, rows, :]), xlayout(x_lo8[:, rows, :])], axis=1
        ).transpose(0, 2, 1, 3, 4))  # [16, 128, 2, 4, 256]
        m2 = np.ascontiguousarray(
            np.broadcast_to(mask_tm[:, rows][:, None, :], (SEQ, 128, RPC))
            .reshape(SEQ // 2, 2, 128, RPC).transpose(0, 2, 1, 3)
        ).astype(bf16)  # [8, 128, 2, 256]
        h0d = np.ascontiguousarray(
            h0[rows].reshape(RPC, 4, 128).transpose(2, 1, 0)).astype(bf16)
        c0d = np.ascontiguousarray(
            c0[rows].reshape(RPC, 4, 128).transpose(2, 1, 0)).astype(bf16)
        in_maps.append({"wh": wh, "wxlo": wxlo, "x8": x8,
                        "m2": m2, "h0": h0d, "c0": c0d})
    return in_maps


def _postprocess(results):
    """Per-core [16,2,128,4,128] bf16 outputs -> full [512, 64, 512] f32."""
    outs = []
    for key in ("hys", "cys"):
        tm = np.empty((SEQ, BATCH, H), dtype=np.float32)
        for cix, res in enumerate(results):
            rows = slice(cix * RPC, (cix + 1) * RPC)
            # res[t, s, p, kc, b'] -> tm[t, 128s+b', 128kc+p]
            r = np.asarray(res[key], dtype=np.float32)
            tm[:, rows, :] = (
                r.transpose(0, 1, 4, 3, 2).reshape(SEQ, RPC, H)
            )
        out = tm.reshape(SEQ, NT, NA, H).transpose(1, 0, 2, 3).reshape(BS, NA, H)
        outs.append(np.ascontiguousarray(out))
    return outs[0], outs[1]


def kernel(h_self, h_inter, hxs, cxs, reset, W_ih, W_hh, b_ih, b_hh, seq_len,
           trace=False, tmpdir=None):
    assert int(seq_len) == SEQ
    from concourse.bass_utils import run_bass_kernel_spmd

    nc = _get_nc()
    in_maps = _prep_inputs(
        np.asarray(h_self), np.asarray(h_inter), np.asarray(hxs), np.asarray(cxs),
        np.asarray(reset), np.asarray(W_ih), np.asarray(W_hh),
        np.asarray(b_ih), np.asarray(b_hh),
    )
    res = run_bass_kernel_spmd(
        nc, in_maps, core_ids=list(range(N_CORES)), trace=trace, tmpdir=tmpdir
    )
    _CACHE["last_results"] = res
    return _postprocess(res.results)


# revision 38
# speedup vs baseline: 1.9279x; 1.0665x over previous
"""Trainium2 Bass kernel for nn_MemoryBlock (batched LSTM scan with reset gating).

Problem (hardcoded shapes):
  bs=512, na=64, seq_len=16, nt=32, H=512, N_ATTN=256.
  x = concat(h_self[:,:,256:], h_inter, -1)            -> [512, 64, 512]
  time-major X: [16, 2048, 512]; LSTM cell per step with
  h,c reset-masked by (1-reset) before the cell. Outputs all
  intermediate h,c states, remapped back to [512, 64, 512].

Strategy: data-parallel over the 2048-row step-batch, 256 rows/core on 8
cores; weights replicated. Per core the batch splits into 2 independent
128-column streams so the recurrence latency of one stream hides behind the
other's engine work.

Matmuls run as fp8e4 DoubleRow (2 K-tiles of 128 per instruction, 0.5
cycles/row): weights are scaled x128 and split hi+residual; per-gate product
counts are chosen by error sensitivity (validated vs the fp32 reference,
rel err ~1.4e-2 < 2e-2):
  g (tanh, slope 1):   x_hi@W_hi + x_lo@W_hi + x_hi@W_lo   (3 products)
  f (sigmoid, mult.):  x_hi@W_hi + x_hi@W_lo                (2 products)
  i, o (sigmoid):      x_hi@W_hi                            (1 product)
  h-part (all gates):  h_fp8@W_hh_hi                        (1 product)
Residuals are stored unscaled (subnormal-heavy but only on small elements,
whose products are negligible). PSUM accumulates fp32; the x128 weight scale
is divided out by the activation's scale=1/128.

Gate order in PSUM is permuted to [g, i, f, o] so each stream evacuates with
two ACT instructions: tanh over the g bank, sigmoid over the i/f/o banks
(3 contiguous banks, one instruction). Cell math and reset-mask muls are
bf16 on DVE (2x_1p); h re-quantizes to fp8 for the next step's matmul.
Outputs stream out as bf16 over the SWDGE path (gpsimd dma_start) so store
descriptor generation stays off the HWDGE ring that feeds the loads; the
host converts back to f32.

Layouts (per core), feature-major "T" = [feature-on-partition, batch]:
  wh   [128, 64, 2, 128] fp8: wh[p, 16*pair+mi, j, q] = A[128*(2pair+j)+p, 128*mi+q]
       A = 128 * [W_ih | W_hh].T with gate columns permuted to (g,i,f,o)
  wxlo [128, 16, 2, 128] fp8: same for the x-row residual of A, gates g,f only
  x8   [16, 128, 2, 4, 256] fp8: x hi/lo terms, x8[t, p, e, kc, b] = X_e[t, row b, 128kc+p]
  m2   [8, 128, 2, 256] bf16: (1-reset) replicated over partitions, step pairs
  h0, c0 [128, 4, 256] bf16: initial states, feature-major
  hys, cys [16, 2, 128, 4, 128] bf16: outputs, stream-major (host transposes back)
"""

import sys

import numpy as np

sys.path.insert(0, "/opt/pypackages")
sys.path.insert(0, "/opt/trn_rl_repo")

import concourse.bass as bass  # noqa: E402,F401
import concourse.bacc as bacc  # noqa: E402
import concourse.mybir as mybir  # noqa: E402
import concourse.tile as tile  # noqa: E402

SEQ = 16
NT = 32
NA = 64
H = 512
N_ATTN = 256
BS = NT * SEQ  # 512
BATCH = NT * NA  # 2048
N_CORES = 8
RPC = BATCH // N_CORES  # 256 rows per core
WS = 128.0  # weight pre-scale, divided out in the activation
F32 = mybir.dt.float32
BF16 = mybir.dt.bfloat16
FP8 = mybir.dt.float8e4
DR = mybir.MatmulPerfMode.DoubleRow

_CACHE = {}


def _build_bass():
    """Build the single-core Bass program (same NEFF runs SPMD on 8 cores)."""
    nc = bacc.Bacc(None, target_bir_lowering=False)

    wh_d = nc.dram_tensor("wh", [128, 64, 2, 128], FP8, kind="ExternalInput")
    wxlo_d = nc.dram_tensor("wxlo", [128, 16, 2, 128], FP8, kind="ExternalInput")
    x8_d = nc.dram_tensor("x8", [SEQ, 128, 2, 4, 256], FP8, kind="ExternalInput")
    m2_d = nc.dram_tensor("m2", [SEQ // 2, 128, 2, 256], BF16, kind="ExternalInput")
    h0_d = nc.dram_tensor("h0", [128, 4, 256], BF16, kind="ExternalInput")
    c0_d = nc.dram_tensor("c0", [128, 4, 256], BF16, kind="ExternalInput")
    hys_d = nc.dram_tensor("hys", [SEQ, 2, 128, 4, 128], BF16, kind="ExternalOutput")
    cys_d = nc.dram_tensor("cys", [SEQ, 2, 128, 4, 128], BF16, kind="ExternalOutput")

    SIG = mybir.ActivationFunctionType.Sigmoid
    TANH = mybir.ActivationFunctionType.Tanh

    with tile.TileContext(nc) as tc:
        with (
            tc.tile_pool(name="const", bufs=1) as const,
            tc.tile_pool(name="xin", bufs=4) as xin,
            tc.tile_pool(name="min", bufs=4) as min_,
            tc.tile_pool(name="state", bufs=2) as state,
            tc.tile_pool(name="gates", bufs=2) as gpool,
            tc.tile_pool(name="psum", bufs=1, space="PSUM") as psum,
        ):
            # --- preamble DMAs, in first-consumption order -----------------
            x_tiles, m_pairs = {}, {}

            def load_x(t):
                a = xin.tile([128, 2, 4, 256], FP8, tag="x8", name=f"x8_{t}")
                nc.sync.dma_start(a[:], x8_d[t])
                x_tiles[t] = a

            def load_m(pair):
                m = min_.tile([128, 2, 256], BF16, tag="m", name=f"m{pair}")
                nc.sync.dma_start(m[:], m2_d[pair])
                m_pairs[pair] = m

            def m_ap(t):
                return m_pairs[t // 2][:, t % 2]

            # preload both ACT function tables (sigmoid + tanh) on a scrap
            # tile so the 1.3us table loads overlap the weight DMA ramp
            scrap = state.tile([128, 1], BF16, tag="scrap", bufs=1)
            nc.vector.memset(scrap[:], 0.0)
            nc.scalar.activation(scrap[:], scrap[:],
                                 mybir.ActivationFunctionType.Sigmoid)
            nc.scalar.activation(scrap[:], scrap[:],
                                 mybir.ActivationFunctionType.Tanh)

            load_x(0)
            # weights: x-row pairs (0,1) first so step-0 x-matmuls start early
            wh = const.tile([128, 64, 2, 128], FP8, tag="wh", name="wh")
            wxlo = const.tile([128, 16, 2, 128], FP8, tag="wxlo", name="wxlo")
            nc.sync.dma_start(wh[:, 0:16], wh_d[:, 0:16])
            load_m(0)
            nc.sync.dma_start(wh[:, 16:32], wh_d[:, 16:32])
            nc.sync.dma_start(wxlo[:], wxlo_d[:])
            h0 = state.tile([128, 4, 256], BF16, tag="h_init", name="h0", bufs=1)
            c0 = state.tile([128, 4, 256], BF16, tag="c_init", name="c0", bufs=1)
            nc.gpsimd.dma_start(h0[:], h0_d[:])
            nc.gpsimd.dma_start(c0[:], c0_d[:])
            nc.sync.dma_start(wh[:, 32:48], wh_d[:, 32:48])
            nc.sync.dma_start(wh[:, 48:64], wh_d[:, 48:64])
            load_x(1)
            load_m(1)
            load_x(2)
            load_x(3)

            def lw(pair, mi):
                return wh[:, 16 * pair + mi]  # [128, 2, 128] fp8

            def lwx(pair, mi):
                # residual weights: g at slots 0-3, f at slots 4-7
                j = mi if mi < 4 else mi - 4
                return wxlo[:, 8 * pair + j]

            # --- per-step PSUM tiles & bank-group bookkeeping --------------
            # per stream, 4 banks: g (1), i+f (2), o (1). Finer tiles give
            # finer WAR deps so next-step x-matmuls start as each evac lands.
            def new_psum(t):
                tiles = []
                for s in range(2):
                    g = psum.tile([128, 4, 128], F32, tag=f"gps{s}",
                                  name=f"gps{t}_{s}")
                    pif = psum.tile([128, 8, 128], F32, tag=f"pif{s}",
                                    name=f"pif{t}_{s}")
                    po = psum.tile([128, 4, 128], F32, tag=f"po{s}",
                                   name=f"po{t}_{s}")
                    tiles.append((g, pif, po))
                return tiles

            def out_slot(tiles, s, mi):
                g, pif, po = tiles[s]
                if mi < 4:
                    return g[:, mi, :], (s, 0)
                if mi < 12:
                    return pif[:, mi - 4, :], (s, 1 + (mi - 4) // 4)
                return po[:, mi - 12, :], (s, 3)

            # --- matmul descriptor builders (flags assigned at flush) ------
            # x products for stream s over mi groups, bank-major (matching
            # the evac order that frees each bank), product-major inside
            def x_descs(tiles, xh_t, xl_t, s, mis):
                cols = slice(128 * s, 128 * (s + 1))
                plan = [(lw, 0, 0, None), (lw, 0, 1, None),
                        (lwx, 0, 0, "gf"), (lwx, 0, 1, "gf"),
                        (lw, 1, 0, "g"), (lw, 1, 1, "g")]
                out = []
                groups = [[mi for mi in mis if mi < 4],
                          [mi for mi in mis if 4 <= mi < 12],
                          [mi for mi in mis if mi >= 12]]
                for grp in groups:
                    for wfn, term, pair, filt in plan:
                        rhs_t = (xh_t, xl_t)[term]
                        for mi in grp:
                            if filt == "g" and mi >= 4:
                                continue
                            if filt == "gf" and not (mi < 4 or 8 <= mi < 12):
                                continue
                            o, bank = out_slot(tiles, s, mi)
                            out.append((o, wfn(pair, mi),
                                        rhs_t[:, 2 * pair : 2 * pair + 2, cols],
                                        bank))
                return out

            # h products for stream s, bank-major so each bank's accumulation
            # completes (stop lands) after only 8 matmuls, unlocking its evac
            def h_descs(tiles, hm, s):
                out = []
                for b in range(4):
                    for pair in (2, 3):
                        for mi in range(4 * b, 4 * b + 4):
                            o, bank = out_slot(tiles, s, mi)
                            out.append((o, lw(pair, mi),
                                        hm[s][:, 2 * (pair - 2) : 2 * pair - 2, :],
                                        bank))
                return out

            # emit a step's matmuls: first write per bank gets start=True,
            # last gets stop=True (PSUM accumulation order is commutative)
            def flush_mms(descs):
                last = {}
                for i, (_, _, _, bank) in enumerate(descs):
                    last[bank] = i
                started = set()
                for i, (o, lhsT, rhs, bank) in enumerate(descs):
                    st = bank not in started
                    started.add(bank)
                    nc.tensor.matmul(o, lhsT, rhs, start=st,
                                     stop=(last[bank] == i), perf_mode=DR)

            def m_bcast(t, s, chunks):
                return (m_ap(t)[:, 128 * s : 128 * (s + 1)]
                        .unsqueeze(1).broadcast_to([128, chunks, 128]))

            # t=0 reset-mask on the initial state (DVE)
            def mask0_s(s):
                hp = h0[:, :, 128 * s : 128 * (s + 1)]
                cp = c0[:, :, 128 * s : 128 * (s + 1)]
                hmv = state.tile([128, 4, 128], FP8, tag=f"hm{s}",
                                 name=f"hm0_{s}")
                cmv = state.tile([128, 4, 128], BF16, tag=f"cm{s}",
                                 name=f"cm0_{s}")
                nc.vector.tensor_mul(hmv[:], hp, m_bcast(0, s, 4))
                nc.vector.tensor_mul(cmv[:], cp, m_bcast(0, s, 4))
                return hmv, cmv

            # cell-evac stage for stream s: evacuate all gates, compute cn,
            # and pre-mask o (om) and cn (cm) with step-(t+1)'s reset mask so
            # the post-tanh critical path is a single multiply.
            def evac_pre(t, tiles, cm, s):
                g_ps, pif_ps, po_ps = tiles[s]
                gt = gpool.tile([128, 4, 128], BF16, tag=f"g{s}",
                                name=f"g{t}_{s}")
                nc.scalar.activation(gt[:], g_ps[:], TANH, scale=1.0 / WS)
                sif = gpool.tile([128, 8, 128], BF16, tag=f"sif{s}",
                                 name=f"sif{t}_{s}")
                nc.scalar.activation(sif[:], pif_ps[:], SIG, scale=1.0 / WS)
                so = gpool.tile([128, 4, 128], BF16, tag=f"so{s}",
                                name=f"so{t}_{s}")
                nc.scalar.activation(so[:], po_ps[:], SIG, scale=1.0 / WS)
                fcm = state.tile([128, 4, 128], BF16, tag=f"fcm{s}",
                                 name=f"fcm{t}_{s}")
                nc.vector.tensor_mul(fcm[:], sif[:, 4:8], cm[s][:])
                ig = state.tile([128, 4, 128], BF16, tag=f"ig{s}",
                                name=f"ig{t}_{s}")
                nc.vector.tensor_mul(ig[:], sif[:, 0:4], gt[:])
                cn = state.tile([128, 4, 128], BF16, tag=f"cn{s}",
                                name=f"c{t}_{s}")
                nc.vector.tensor_add(cn[:], ig[:], fcm[:])
                nc.sync.dma_start(cys_d[t, s], cn[:])
                om = cmn = None
                if t + 1 < SEQ:
                    om = state.tile([128, 4, 128], BF16, tag=f"om{s}",
                                    name=f"om{t}_{s}")
                    nc.vector.tensor_mul(om[:], so[:], m_bcast(t + 1, s, 4))
                    cmn = state.tile([128, 4, 128], BF16, tag=f"cm{s}",
                                     name=f"cm{t + 1}_{s}")
                    nc.vector.tensor_mul(cmn[:], cn[:], m_bcast(t + 1, s, 4))
                return so, cn, om, cmn

            # post-tanh stage for stream s: tanh(c); hm(t+1) first (critical:
            # feeds next h-matmuls), then h for the output store.
            def cell_post(t, so, cn, om, s):
                th = state.tile([128, 4, 128], BF16, tag=f"th{s}",
                                name=f"th{t}_{s}")
                nc.scalar.activation(th[:], cn[:], TANH)
                hmn = None
                if om is not None:
                    hmn = state.tile([128, 4, 128], FP8, tag=f"hm{s}",
                                     name=f"hm{t + 1}_{s}")
                    nc.vector.tensor_mul(hmn[:], om[:], th[:])
                hn = state.tile([128, 4, 128], BF16, tag=f"hn{s}",
                                name=f"h{t}_{s}")
                nc.vector.tensor_mul(hn[:], so[:], th[:])
                # last step's stores ride the faster HWDGE gen (idle at tail)
                eng = nc.sync if t == SEQ - 1 else nc.gpsimd
                eng.dma_start(hys_d[t, s], hn[:])
                return hmn

            # --- t=0: masks on initial state -------------------------------
            # touch m0 with a 1-elem DVE copy so the first mask-mul carries
            # a single DMA sem wait (walrus allows one sync wait per instr).
            tch = state.tile([128, 1], BF16, tag="tch", bufs=1)
            nc.vector.tensor_copy(tch[:], m_pairs[0][:, 0, :1])
            cur_psum = new_psum(0)
            hm0a, cm0a = mask0_s(0)
            hm0b, cm0b = mask0_s(1)
            hm, cm = [hm0a, hm0b], [cm0a, cm0b]
            xt0 = x_tiles.pop(0)
            flush_mms(
                x_descs(cur_psum, xt0[:, 0], xt0[:, 1], 0, range(16))
                + x_descs(cur_psum, xt0[:, 0], xt0[:, 1], 1, range(16))
                + h_descs(cur_psum, hm, 0)
                + h_descs(cur_psum, hm, 1)
            )

            for t in range(SEQ):
                # (this step's matmuls were emitted in the previous iteration)
                # ACT order: g_A sif_A so_A | g_B th_A | sif_B so_B th_B —
                # th_A slots after g_B exactly when cn_A lands; B's DVE ops
                # are emitted after A's th-dependent ops so the in-order DVE
                # queue never head-blocks on the other stream.
                so_a, cn_a, om_a, cm_a = evac_pre(t, cur_psum, cm, 0)
                # emit stream-B's g evac before th_A on ACT
                g_ps, pif_ps, po_ps = cur_psum[1]
                gt_b = gpool.tile([128, 4, 128], BF16, tag="g1",
                                  name=f"g{t}_1")
                nc.scalar.activation(gt_b[:], g_ps[:], TANH, scale=1.0 / WS)
                hm_a = cell_post(t, so_a, cn_a, om_a, 0)
                # stream B evacs (g already emitted) + cell
                sif_b = gpool.tile([128, 8, 128], BF16, tag="sif1",
                                   name=f"sif{t}_1")
                nc.scalar.activation(sif_b[:], pif_ps[:], SIG, scale=1.0 / WS)
                so_b = gpool.tile([128, 4, 128], BF16, tag="so1",
                                  name=f"so{t}_1")
                nc.scalar.activation(so_b[:], po_ps[:], SIG, scale=1.0 / WS)
                fcm_b = state.tile([128, 4, 128], BF16, tag="fcm1",
                                   name=f"fcm{t}_1")
                nc.vector.tensor_mul(fcm_b[:], sif_b[:, 4:8], cm[1][:])
                ig_b = state.tile([128, 4, 128], BF16, tag="ig1",
                                  name=f"ig{t}_1")
                nc.vector.tensor_mul(ig_b[:], sif_b[:, 0:4], gt_b[:])
                cn_b = state.tile([128, 4, 128], BF16, tag="cn1",
                                  name=f"c{t}_1")
                nc.vector.tensor_add(cn_b[:], ig_b[:], fcm_b[:])
                nc.sync.dma_start(cys_d[t, 1], cn_b[:])
                om_b = cm_b = None
                if t + 1 < SEQ:
                    om_b = state.tile([128, 4, 128], BF16, tag="om1",
                                      name=f"om{t}_1")
                    nc.vector.tensor_mul(om_b[:], so_b[:], m_bcast(t + 1, 1, 4))
                    cm_b = state.tile([128, 4, 128], BF16, tag="cm1",
                                      name=f"cm{t + 1}_1")
                    nc.vector.tensor_mul(cm_b[:], cn_b[:], m_bcast(t + 1, 1, 4))
                hm_b = cell_post(t, so_b, cn_b, om_b, 1)
                if t + 1 < SEQ:
                    hm, cm = [hm_a, hm_b], [cm_a, cm_b]
                    if t + 4 < SEQ:
                        load_x(t + 4)
                    if t % 2 == 0 and (t + 4) // 2 < SEQ // 2:
                        load_m((t + 4) // 2)
                    nxt = new_psum(t + 1)
                    xt = x_tiles.pop(t + 1)
                    xh_t, xl_t = xt[:, 0], xt[:, 1]
                    # PE emission for step t+1, ordered by when inputs/banks
                    # become available: all of stream A's x products, B's
                    # g-chunk x products, A's h products (hm_A lands after
                    # th_A), B's remaining x (gated on B's late evacs), and
                    # B's h products last (hm_B comes from th_B).
                    flush_mms(
                        x_descs(nxt, xh_t, xl_t, 0, range(16))
                        + x_descs(nxt, xh_t, xl_t, 1, range(4))
                        + h_descs(nxt, hm, 0)
                        + x_descs(nxt, xh_t, xl_t, 1, range(4, 16))
                        + h_descs(nxt, hm, 1)
                    )
                    cur_psum = nxt

    nc.compile()
    return nc


def _get_nc():
    if "nc" not in _CACHE:
        _CACHE["nc"] = _build_bass()
    return _CACHE["nc"]


def _prep_inputs(h_self, h_inter, hxs, cxs, reset, W_ih, W_hh, b_ih, b_hh):
    """Host-side layout transforms -> list of per-core input dicts."""
    import ml_dtypes

    f = np.float32
    F8 = ml_dtypes.float8_e4m3
    bf16 = ml_dtypes.bfloat16

    x = np.concatenate([h_self[:, :, N_ATTN:], h_inter], axis=-1).astype(f)
    x_tm = np.ascontiguousarray(
        x.reshape(NT, SEQ, NA, H).transpose(1, 0, 2, 3).reshape(SEQ, BATCH, H)
    )
    resets = np.broadcast_to(reset.astype(f), (BS, NA))
    resets_tm = resets.reshape(NT, SEQ, NA).transpose(1, 0, 2).reshape(SEQ, BATCH)
    mask_tm = (1.0 - resets_tm).astype(f)
    h0 = hxs[::SEQ].reshape(BATCH, H).astype(f)
    c0 = cxs[::SEQ].reshape(BATCH, H).astype(f)

    assert not np.any(b_ih) and not np.any(b_hh), "nonzero LSTM bias unsupported"

    # A = 128 * [W_ih | W_hh].T [1024, 2048], gate columns permuted to g,i,f,o
    A = (np.concatenate([W_ih, W_hh], axis=1).T.astype(f) * WS)  # [1024, 2048]
    perm = np.concatenate([np.arange(2 * H, 3 * H),      # g
                           np.arange(0, H),              # i
                           np.arange(H, 2 * H),          # f
                           np.arange(3 * H, 4 * H)])     # o
    A = A[:, perm]
    A_hi8 = A.astype(F8)
    A_hi = A_hi8.astype(f)
    A_xlo = A[:H] - A_hi[:H]  # x rows residual
    # wh[p, 16*pair+mi, j, q] = A_hi[128*(2pair+j)+p, 128mi+q]
    wh = np.ascontiguousarray(
        A_hi8.reshape(4, 2, 128, 16, 128).transpose(2, 0, 3, 1, 4)
        .reshape(128, 64, 2, 128)
    )
    # wxlo: x pairs (0,1) x gate chunks g (0-3) and f (8-11)
    gf = np.concatenate([A_xlo[:, 0:512], A_xlo[:, 1024:1536]], axis=1)
    wxlo = np.ascontiguousarray(
        gf.astype(F8).reshape(2, 2, 128, 8, 128).transpose(2, 0, 3, 1, 4)
        .reshape(128, 16, 2, 128)
    )

    x_hi8 = x_tm.astype(F8)
    x_lo8 = (x_tm - x_hi8.astype(f)).astype(F8)

    def xlayout(a):  # [16, RPC rows, 512] -> [16, 128, 4, 256]
        return a.reshape(SEQ, RPC, 4, 128).transpose(0, 3, 2, 1)

    in_maps = []
    for cix in range(N_CORES):
        rows = slice(cix * RPC, (cix + 1) * RPC)
        x8 = np.ascontiguousarray(np.stack(
            [xlayout(x_hi8[:, rows, :]), xlayout(x_lo8[:, rows, :])], axis=1
        ).transpose(0, 2, 1, 3, 4))  # [16, 128, 2, 4, 256]
        m2 = np.ascontiguousarray(
            np.broadcast_to(mask_tm[:, rows][:, None, :], (SEQ, 128, RPC))
            .reshape(SEQ // 2, 2, 128, RPC).transpose(0, 2, 1, 3)
        ).astype(bf16)  # [8, 128, 2, 256]
        h0d = np.ascontiguousarray(
            h0[rows].reshape(RPC, 4, 128).transpose(2, 1, 0)).astype(bf16)
        c0d = np.ascontiguousarray(
            c0[rows].reshape(RPC, 4, 128).transpose(2, 1, 0)).astype(bf16)
        in_maps.append({"wh": wh, "wxlo": wxlo, "x8": x8,
                        "m2": m2, "h0": h0d, "c0": c0d})
    return in_maps


def _postprocess(results):
    """Per-core [16,2,128,4,128] bf16 outputs -> full [512, 64, 512] f32."""
    outs = []
    for key in ("hys", "cys"):
        tm = np.empty((SEQ, BATCH, H), dtype=np.float32)
        for cix, res in enumerate(results):
            rows = slice(cix * RPC, (cix + 1) * RPC)
            # res[t, s, p, kc, b'] -> tm[t, 128s+b', 128kc+p]
            r = np.asarray(res[key], dtype=np.float32)
            tm[:, rows, :] = (
                r.transpose(0, 1, 4, 3, 2).reshape(SEQ, RPC, H)
            )
        out = tm.reshape(SEQ, NT, NA, H).transpose(1, 0, 2, 3).reshape(BS, NA, H)
        outs.append(np.ascontiguousarray(out))
    return outs[0], outs[1]


def kernel(h_self, h_inter, hxs, cxs, reset, W_ih, W_hh, b_ih, b_hh, seq_len,
           trace=False, tmpdir=None):
    assert int(seq_len) == SEQ
    from concourse.bass_utils import run_bass_kernel_spmd

    nc = _get_nc()
    in_maps = _prep_inputs(
        np.asarray(h_self), np.asarray(h_inter), np.asarray(hxs), np.asarray(cxs),
        np.asarray(reset), np.asarray(W_ih), np.asarray(W_hh),
        np.asarray(b_ih), np.asarray(b_hh),
    )
    res = run_bass_kernel_spmd(
        nc, in_maps, core_ids=list(range(N_CORES)), trace=trace, tmpdir=tmpdir
    )
    _CACHE["last_results"] = res
    return _postprocess(res.results)
